# revision 1
# baseline (speedup 1.0000x reference)
"""CMC-V2 loss kernel for 8 Trainium2 NeuronCores (Bass/Tile).

Math
----
The reference loss decomposes into:
  - 9 NT-Xent contrastive terms. For pair (A, B) with row-normalized
    embeddings Z = [An; Bn] (N=4096 rows, D=512), the per-row sim matrix is
    sim = (Zn @ Zn.T)/0.2 = 5*cos.  Since rows are unit-norm, sim[i,i] = 5.0
    is the exact row max, so
        lse_i (diag excluded) = 5 + log(S_i - 1),  S_i = sum_j exp(5*cos_ij - 5)
    and sum_i pos_i = 10 * sum_i cos(An_i, Bn_i).
    per-pair loss = 5 + (1/4096) sum_i log(S_i - 1) - (10/4096) sum_i cos_i
  - 12 cosine-embedding terms: 1 - (1/2048) sum_i cos_i.
  Total constant: 9*5 + 12 = 57.

Sharding
--------
Data-parallel over 8 cores with a static SPMD program: core c receives every
input rolled by -256*c rows, so its shard is always rows [0:256) of each
matrix (matmul weights cannot take dynamic offsets). Each core:
  - normalizes all 12 half-matrices (bf16) and transposes them on the PE
    (identity matmul -> PSUM -> ScalarE/DVE copy) into ZnT layout
    [128 part = d%128, 4 = d//128, 2048 = sample],
  - computes its 512 Gram rows per pair (lhsT = its 256-row shard of A and B)
    against all 4096 columns; ScalarE applies exp(5x-5) with a fused
    per-row accumulate; log(S-1) summed on-chip,
  - computes its 256-row shard of the 21 row-dot (cosine) sums,
  - returns partial sums in a [128, 4] f32 tensor.
Host sums the 8 partials and applies the closed-form combination.
"""

import numpy as np
import ml_dtypes
from contextlib import ExitStack

from concourse import bass, bacc, tile, mybir
from concourse.bass_utils import run_bass_kernel_spmd

BF16 = mybir.dt.bfloat16
FP8 = mybir.dt.float8e4
F32 = mybir.dt.float32
AF = mybir.ActivationFunctionType
ALU = mybir.AluOpType

# fp8 variant: Gram matmuls in fp8e4m3 with DoubleRow (2 MACs/cell/cycle).
# Normalized rows are pre-scaled by 16 so fp8 sees values ~N(0, 0.71^2);
# the Gram then yields 256*cos and the exp scale becomes 5/256.
USE_FP8 = True
FP8_SCALE = 16.0

B = 2048          # batch
DH = 512          # half feature dim
N_CORES = 8
R = B // N_CORES  # 256 rows per core shard
NT = B // 128     # 16 row tiles per half-matrix
KC = DH // 128    # 4 contraction chunks
CBW = 512         # column block width
CB = B // CBW     # 4 col blocks per matrix

NAMES = ["f1_m0", "f1_m1", "f1_m2", "f2_m0", "f2_m1", "f2_m2"]

# contrastive pairs as ((f, h), (f, h)); h: 0 = shared, 1 = private
PAIRS_S1 = [((0, 0), (1, 0)), ((0, 0), (2, 0)), ((1, 0), (2, 0))]
PAIRS_S2 = [((3, 0), (4, 0)), ((3, 0), (5, 0)), ((4, 0), (5, 0))]
PAIRS_P = [((0, 1), (3, 1)), ((1, 1), (4, 1)), ((2, 1), (5, 1))]
ORTHO_V1 = [((0, 0), (0, 1)), ((1, 0), (1, 1)), ((2, 0), (2, 1)),
            ((0, 1), (1, 1)), ((0, 1), (2, 1)), ((1, 1), (2, 1))]
ORTHO_V2 = [((3, 0), (3, 1)), ((4, 0), (4, 1)), ((5, 0), (5, 1)),
            ((3, 1), (4, 1)), ((3, 1), (5, 1)), ((4, 1), (5, 1))]

N_SLOTS = 9 * 4   # 9 pairs x 4 M-tiles of 128 Gram rows each
N_DOTS = 21       # 9 contrastive + 12 ortho row-dot sums



def build_program(use_fp8=USE_FP8, repeat=1, loads_on="gpsimd",
                  psum_banks=2, psum_bufs=3, timing_mode="full",
                  transpose_via="pe", squares_on="vector", copy_mod=2):
    # Restrict ACT table selection to the one set containing BOTH exp and ln
    # (greedy per-op selection would otherwise thrash exp_and_others <->
    # natural_log, ~1.3us per reload, serialized on ScalarE).
    if not getattr(bacc, "_ant_act_tables_patched", False):
        _orig_tables = bacc.get_activation_tables

        def _patched(arch):
            tabs = _orig_tables(arch)
            return {k: (v if k == "natural_log_exp_and_others" else set())
                    for k, v in tabs.items()}

        bacc.get_activation_tables = _patched
        bacc._ant_act_tables_patched = True

    nc = bacc.Bacc(
        "TRN2",
        target_bir_lowering=False,
        debug=False,
        enable_asserts=False,
        num_devices=N_CORES,
    )
    ffs = [nc.dram_tensor(n, [B, 2 * DH], BF16, kind="ExternalInput").ap()
           for n in NAMES]
    out_dram = nc.dram_tensor("part", [128, 4], F32, kind="ExternalOutput").ap()

    n_sub = 2 * B // (psum_banks * CBW)     # psum tiles per (pair, mtile)
    cb_per = psum_banks                     # 512-col blocks per psum tile

    with tile.TileContext(nc) as tc, ExitStack() as ctx:
        znt_pool = ctx.enter_context(tc.tile_pool(name="zntp", bufs=9))
        x_pool = ctx.enter_context(tc.tile_pool(name="xp", bufs=6))
        zn_pool = ctx.enter_context(tc.tile_pool(name="znp", bufs=4))
        vscr_pool = ctx.enter_context(tc.tile_pool(name="vscrp", bufs=3))
        escr_pool = ctx.enter_context(tc.tile_pool(name="escrp", bufs=3))
        nrm_pool = ctx.enter_context(tc.tile_pool(name="nrmp", bufs=3))
        sab_pool = ctx.enter_context(tc.tile_pool(name="sabp", bufs=4))
        acc_pool = ctx.enter_context(tc.tile_pool(name="accp", bufs=1))
        psum_pool = ctx.enter_context(
            tc.tile_pool(name="psump", bufs=psum_bufs, space="PSUM"))

        load_eng = {"gpsimd": nc.gpsimd, "scalar": nc.scalar,
                    "sync": nc.sync}[loads_on]
        # xbar transposes alternate across both HWDGE rings (SP + ACT) so
        # their trigger/ucode cost is not serialized on one ring.
        tr_engs = [nc.sync, nc.scalar]
        tr_i = [0]

        def tr_dma(**kwargs):
            tr_engs[tr_i[0] % 2].dma_start(**kwargs)
            tr_i[0] += 1

        biasm5 = acc_pool.tile([128, 1], F32, tag="biasm5", name="biasm5")
        nc.gpsimd.memset(biasm5[:], -5.0)
        if transpose_via == "pe":
            # identity for PE transposes: ident[p, j] = (j == p)
            ident = acc_pool.tile([128, 128], BF16, tag="ident", name="ident")
            iota_r = acc_pool.tile([128, 128], F32, tag="iota_r", name="iota_r")
            iota_p = acc_pool.tile([128, 1], F32, tag="iota_p", name="iota_p")
            nc.gpsimd.iota(iota_r[:], pattern=[[1, 128]], base=0,
                           channel_multiplier=0,
                           allow_small_or_imprecise_dtypes=True)
            nc.gpsimd.iota(iota_p[:], pattern=[[0, 1]], base=0,
                           channel_multiplier=1,
                           allow_small_or_imprecise_dtypes=True)
            nc.vector.tensor_scalar(
                out=ident[:], in0=iota_r[:], scalar1=iota_p[:, 0:1],
                scalar2=None, op0=ALU.is_equal)
        cp_i = [0]
        sm1 = acc_pool.tile([128, N_SLOTS], F32, tag="sm1", name="sm1")
        dots_all = acc_pool.tile([128, N_DOTS], F32, tag="dots", name="dots_all")
        logv = acc_pool.tile([128, N_SLOTS], F32, tag="logv", name="logv")
        part = acc_pool.tile([128, 4], F32, tag="part", name="part_sb")

        znt = {}

        rep_ctx = tc.For_i(0, repeat, 1) if repeat > 1 else None
        if rep_ctx is not None:
            rep_ctx.__enter__()

        def build_ff(f):
            """Load ff tensor f once per row-tile; normalize both halves and
            store transposed (bf16 Zn, or 16*Zn cast to fp8e4m3).
            znt[(f,h)][p, c, j] = Zn_h[j, c*128 + p]."""
            zts = []
            for h in range(2):
                # fp8: the PSUM-drain copy casts bf16->fp8 directly, so the
                # znt tile is fp8 from the start (no intermediate + cast pass)
                zts.append(znt_pool.tile(
                    [128, KC, B], FP8 if use_fp8 else BF16, tag="znt",
                    name=f"znt{f}_{h}"))
            norms = nrm_pool.tile([128, 2 * NT], F32, tag="norms", name=f"nrm{f}")
            lgn = nrm_pool.tile([128, 2 * NT], F32, tag="lgn", name=f"lgn{f}")
            rinv = nrm_pool.tile([128, 2 * NT], F32, tag="rinv", name=f"rinv{f}")
            for g in range(NT // 4):
                xts = []
                for u in range(2):
                    # one 3D DMA covers two 128-row tiles: [128, 2, 1024]
                    xt = x_pool.tile([128, 2, 2 * DH], BF16, tag="xt",
                                     name=f"xt{f}_{g}_{u}")
                    base = (4 * g + 2 * u) * 128
                    load_eng.dma_start(
                        out=xt[:],
                        in_=ffs[f][base:base + 256, :].rearrange(
                            "(tt p) c -> p tt c", p=128))
                    xts.append(xt)
                for i, t in enumerate(range(4 * g, 4 * g + 4)):
                    xv = xts[i // 2][:, i % 2, :]
                    for h in range(2):
                        sq = vscr_pool.tile([128, DH], F32, tag="vscr",
                                            name=f"sq{f}_{h}_{t}")
                        sq_eng = (nc.gpsimd if squares_on == "gpsimd"
                                  else nc.vector)
                        sq_eng.scalar_tensor_tensor(
                            out=sq[:], in0=xv[:, h * DH:(h + 1) * DH],
                            scalar=1.0, in1=xv[:, h * DH:(h + 1) * DH],
                            op0=ALU.mult, op1=ALU.mult,
                            accum_out=norms[:, h * NT + t:h * NT + t + 1])
                for h in range(2):
                    cs = slice(h * NT + 4 * g, h * NT + 4 * g + 4)
                    # rinv = ss**-0.5 = exp(-0.5*ln(ss)); both funcs live in
                    # the natural_log_exp_and_others table set.
                    nc.scalar.activation(lgn[:, cs], norms[:, cs], AF.Ln)
                    nc.scalar.activation(rinv[:, cs], lgn[:, cs], AF.Exp,
                                         scale=-0.5)
                for i, t in enumerate(range(4 * g, 4 * g + 4)):
                    xv = xts[i // 2][:, i % 2, :]
                    for h in range(2):
                        zn = zn_pool.tile([128, DH], BF16, tag="zn",
                                          name=f"zn{f}_{h}_{t}")
                        if use_fp8:
                            nc.vector.tensor_scalar(
                                out=zn[:], in0=xv[:, h * DH:(h + 1) * DH],
                                scalar1=rinv[:, h * NT + t:h * NT + t + 1],
                                scalar2=FP8_SCALE, op0=ALU.mult, op1=ALU.mult)
                        else:
                            nc.vector.tensor_scalar_mul(
                                out=zn[:], in0=xv[:, h * DH:(h + 1) * DH],
                                scalar1=rinv[:, h * NT + t:h * NT + t + 1])
                        if transpose_via == "xbar":
                            tr_dma(
                                out=zts[h][:, :, t * 128:(t + 1) * 128],
                                in_=zn[:], transpose=True)
                        else:
                            # PE transpose into PSUM, then one strided
                            # PSUM->SBUF copy alternating ScalarE/DVE.
                            tp = psum_pool.tile([128, KC, 128], BF16,
                                                tag="tpp", bufs=2,
                                                name=f"tp{f}_{h}_{t}")
                            for c in range(KC):
                                nc.tensor.transpose(
                                    tp[:, c, :], zn[:, c * 128:(c + 1) * 128],
                                    ident[:])
                            dst = zts[h][:, :, t * 128:(t + 1) * 128]
                            # copy_mod: every copy_mod-th copy on ScalarE,
                            # rest on DVE (0 = all DVE)
                            if copy_mod and cp_i[0] % copy_mod == 0:
                                nc.scalar.copy(dst, tp[:, :, :])
                            else:
                                nc.vector.tensor_copy(dst, tp[:, :, :])
                            cp_i[0] += 1
            for h in range(2):
                znt[(f, h)] = zts[h]

        slot_i = [0]

        def gram(A, Bm):
            """Gram rows + fused exp/rowsum for contrastive pair (A, Bm)."""
            for X in (A, Bm):          # lhsT source (core 256-row shard)
                for mt in range(2):    # two 128-row M tiles
                    si = slot_i[0]
                    sab = sab_pool.tile([128, n_sub], F32, tag="sab",
                                        name=f"sab{si}")
                    for ridx, RH in enumerate((A, Bm)):   # rhs matrix
                        for sub in range(n_sub // 2):
                            ps = psum_pool.tile(
                                [128, cb_per, CBW], F32, tag="gram",
                                name=f"ps{si}_{ridx}_{sub}")
                            for cbl in range(cb_per):
                                cb = sub * cb_per + cbl
                                if use_fp8:
                                    for q in range(KC // 2):
                                        nc.tensor.matmul(
                                            ps[:, cbl, :],
                                            znt[X][:, 2 * q:2 * q + 2,
                                                   mt * 128:(mt + 1) * 128],
                                            znt[RH][:, 2 * q:2 * q + 2,
                                                    cb * CBW:(cb + 1) * CBW],
                                            perf_mode=mybir.MatmulPerfMode.DoubleRow,
                                            start=(q == 0),
                                            stop=(q == KC // 2 - 1))
                                else:
                                    for kc in range(KC):
                                        nc.tensor.matmul(
                                            ps[:, cbl, :],
                                            znt[X][:, kc,
                                                   mt * 128:(mt + 1) * 128],
                                            znt[RH][:, kc,
                                                    cb * CBW:(cb + 1) * CBW],
                                            start=(kc == 0),
                                            stop=(kc == KC - 1))
                            es = escr_pool.tile([128, cb_per, CBW], BF16,
                                                tag="escr",
                                                name=f"es{si}_{ridx}_{sub}")
                            exp_scale = (5.0 / (FP8_SCALE * FP8_SCALE)
                                         if use_fp8 else 5.0)
                            col = ridx * (n_sub // 2) + sub
                            nc.scalar.activation(
                                es[:], ps[:], AF.Exp, bias=biasm5[:],
                                scale=exp_scale,
                                accum_out=sab[:, col:col + 1])
                    # sm1[:, slot] = sum(sab) - 1
                    scr2 = sab_pool.tile([128, n_sub], F32, tag="scr2",
                                         name=f"scr2_{si}")
                    nc.vector.tensor_scalar(
                        out=scr2[:], in0=sab[:], scalar1=-1.0 / n_sub,
                        scalar2=None, op0=ALU.add, op1=ALU.add,
                        accum_out=sm1[:, si:si + 1])
                    slot_i[0] += 1

        def dots(col, X, Y):
            """dots_all[:, col] = per-partition sum over the core 256-row
            shard of <Zn_X[i], Zn_Y[i]> (row-wise cosines)."""
            o = vscr_pool.tile([128, KC, R], F32, tag="vscr", name=f"do{col}")
            dscale = 1.0 / (FP8_SCALE * FP8_SCALE) if use_fp8 else 1.0
            nc.vector.scalar_tensor_tensor(
                out=o[:], in0=znt[X][:, :, 0:R], scalar=dscale,
                in1=znt[Y][:, :, 0:R], op0=ALU.mult, op1=ALU.mult,
                accum_out=dots_all[:, col:col + 1])

        if timing_mode == "grams":
            # timing probe: skip builds; map the 12 halves onto 9 shared
            # tiles (timing-equivalent op stream, garbage data).
            shared = []
            for s in range(9):
                t = znt_pool.tile([128, KC, B], FP8 if use_fp8 else BF16,
                                  tag="znt", name=f"znts{s}")
                nc.vector.memset(t[:, :, 0:2], 0.0)
                shared.append(t)
            for f in range(6):
                for h in range(2):
                    znt[(f, h)] = shared[(2 * f + h) % 9]
            def build_ff(f):
                pass
        elif timing_mode == "builds":
            nc.vector.memset(sm1[:], 1.0)
            def gram(A, Bm):
                pass

        # Interleaved emission: each gram/dot is emitted as soon as the
        # matrices it needs are built, so no engine queue convoys behind an
        # unrelated phase.  dots_all cols: 0..8 contrastive in the order
        # (s1 x3, private x3, s2 x3); 9..20 ortho (v1 then v2).
        build_ff(0)
        build_ff(1)
        gram((0, 0), (1, 0))
        dots(0, (0, 0), (1, 0))
        dots(9, (0, 0), (0, 1)); dots(10, (1, 0), (1, 1))
        dots(12, (0, 1), (1, 1))
        build_ff(2)
        gram((0, 0), (2, 0)); gram((1, 0), (2, 0))
        dots(1, (0, 0), (2, 0)); dots(2, (1, 0), (2, 0))
        dots(11, (2, 0), (2, 1))
        dots(13, (0, 1), (2, 1)); dots(14, (1, 1), (2, 1))
        build_ff(3)
        gram((0, 1), (3, 1))
        dots(3, (0, 1), (3, 1))
        dots(15, (3, 0), (3, 1))
        build_ff(4)
        gram((1, 1), (4, 1)); gram((3, 0), (4, 0))
        dots(4, (1, 1), (4, 1)); dots(6, (3, 0), (4, 0))
        dots(16, (4, 0), (4, 1)); dots(18, (3, 1), (4, 1))
        build_ff(5)
        gram((2, 1), (5, 1)); gram((3, 0), (5, 0)); gram((4, 0), (5, 0))
        dots(5, (2, 1), (5, 1)); dots(7, (3, 0), (5, 0))
        dots(8, (4, 0), (5, 0))
        dots(17, (5, 0), (5, 1)); dots(19, (3, 1), (5, 1))
        dots(20, (4, 1), (5, 1))

        # ---- epilogue ----
        nc.scalar.activation(logv[:], sm1[:], AF.Ln)
        nc.vector.memset(part[:], 0.0)
        nc.vector.tensor_reduce(part[:, 0:1], logv[:], axis=mybir.AxisListType.X,
                                op=ALU.add)
        nc.vector.tensor_reduce(part[:, 1:2], dots_all[:, 0:9],
                                axis=mybir.AxisListType.X, op=ALU.add)
        nc.vector.tensor_reduce(part[:, 2:3], dots_all[:, 9:21],
                                axis=mybir.AxisListType.X, op=ALU.add)
        nc.sync.dma_start(out=out_dram, in_=part[:])

        if rep_ctx is not None:
            rep_ctx.__exit__(None, None, None)

    nc.compile()
    return nc


_PROG = None


def _get_prog():
    global _PROG
    if _PROG is None:
        _PROG = build_program()
    return _PROG


def make_in_maps(inputs):
    bf = ml_dtypes.bfloat16
    in_maps = []
    for c in range(N_CORES):
        m = {}
        for n in NAMES:
            a = np.asarray(inputs[n], dtype=np.float32)
            m[n] = np.ascontiguousarray(np.roll(a, -R * c, axis=0)).astype(bf)
        in_maps.append(m)
    return in_maps


def combine(parts):
    """parts: list of 8 [128, 4] f32 arrays -> scalar loss."""
    tl = tcc = toc = 0.0
    for p in parts:
        p = np.asarray(p, dtype=np.float64)
        tl += p[:, 0].sum()
        tcc += p[:, 1].sum()
        toc += p[:, 2].sum()
    n2 = float(2 * B)
    loss = (9 * 5.0 + 12.0) + tl / n2 - 10.0 * tcc / n2 - toc / float(B)
    return np.float32(loss)


def kernel(**inputs):
    nc = _get_prog()
    in_maps = make_in_maps(inputs)
    res = run_bass_kernel_spmd(nc, in_maps, list(range(N_CORES)))
    return combine([res.results[c]["part"] for c in range(N_CORES)])



# revision 49
# speedup vs baseline: 1.4826x; 1.4826x over previous
"""CMC-V2 loss kernel for 8 Trainium2 NeuronCores (Bass/Tile), v2.

Math
----
Same decomposition as before: 9 NT-Xent terms reduce to
  per-pair loss = 5 + (1/4096) sum_i log(S_i - 1) - (10/4096) sum_i cos_i,
  S_i = sum_j exp(5*cos_ij - 5)   (self term included, the -1 removes it),
plus 12 cosine-embedding terms (1 - mean cos).  Constant 9*5 + 12 = 57.

v2 exploits that exp(sim) is SYMMETRIC and that the 9 pairs share
sub-Grams: the N x N pair matrix splits into per-half "diag" sub-Grams
(AA: 12 of them, shared across pairs) and "rect" sub-Grams (AB: 9).
Each [128,128] block is computed ONCE; its row sums feed S for its rows
and its column sums (ones-matmul over exp, partition reduction) feed S
for its columns.  Per core: diag = 2 row-tiles x 9 col-tiles (offsets
0..8 in rolled space; colsums for offsets 1..7), rect = 2 row-tiles x 16
col-tiles (colsums for all).  504 blocks/core vs 1152 in v1.
S is assembled on the HOST from per-core row/col partial sums (log and
final reduction are O(9*4096) host work, like the existing combine).

Build
-----
Host sends roll(X,-256c).T as [1024,2048] fp8e4m3 (layout+dtype prep).
Per half: squares (fp8, ACT/DVE split) -> norm matmul with an all-ones
fp8 [128,P=128] lhsT so the column norms land REPLICATED across all
partitions in PSUM -> ACT Ln -> ACT Exp(-0.5*ln + ln16) gives the
broadcast rinv tile [128,2048] bf16 directly (no reshapes, no DRAM
roundtrips, no broadcast matmul) -> one in-place DVE multiply makes
znt = (16*Zn)^T in fp8.  Dots run on GpSimd.
"""

import numpy as np
import ml_dtypes
from contextlib import ExitStack

from concourse import bass, bacc, tile, mybir
from concourse.bass_utils import run_bass_kernel_spmd

BF16 = mybir.dt.bfloat16
FP8 = mybir.dt.float8e4
F32 = mybir.dt.float32
AF = mybir.ActivationFunctionType
ALU = mybir.AluOpType

FP8_SCALE = 16.0
LN_SCALE = float(np.log(FP8_SCALE))
EXP_SCALE = 5.0 / (FP8_SCALE * FP8_SCALE)
DSCALE = 1.0 / (FP8_SCALE * FP8_SCALE)

B = 2048          # batch
DH = 512          # half feature dim
N_CORES = 8
R = B // N_CORES  # 256 rows per core shard
KC = DH // 128    # 4 contraction chunks per half
NT = B // 128     # 16 col tiles per matrix

# half ids: 2*f + h  (h=0 shared, h=1 private)
NAMES = ["f1_m0", "f1_m1", "f1_m2", "f2_m0", "f2_m1", "f2_m2"]
PAIRS = [(0, 2), (0, 4), (2, 4),      # shared view1
         (6, 8), (6, 10), (8, 10),    # shared view2
         (1, 7), (3, 9), (5, 11)]     # private cross-view
ORTHO = [(0, 1), (2, 3), (4, 5), (1, 3), (1, 5), (3, 5),
         (6, 7), (8, 9), (10, 11), (7, 9), (7, 11), (9, 11)]

N_RSAB = 9 * 2 * 2 + 12 * 2 * 2   # rect (sub,t,2 chunks) + diag (fh,t,A/B)


def build_program(repeat=1, loads_on="gpsimd", timing_mode="full",
                  sq_pat="A", dots_on="vector", gram_bufs=2, lag=0,
                  bbufs=2, mult_mode="inplace", cs_drain="vector",
                  norm_path="replicated"):
    # Restrict ACT table selection to the set with exp AND ln (square is in
    # every set); avoids ~2.7us table reloads on ScalarE.
    if not getattr(bacc, "_ant_act_tables_patched", False):
        _orig_tables = bacc.get_activation_tables

        def _patched(arch):
            tabs = _orig_tables(arch)
            return {k: (v if k == "natural_log_exp_and_others" else set())
                    for k, v in tabs.items()}

        bacc.get_activation_tables = _patched
        bacc._ant_act_tables_patched = True

    nc = bacc.Bacc(
        "TRN2",
        target_bir_lowering=False,
        debug=False,
        enable_asserts=False,
        num_devices=N_CORES,
    )
    ffs = [nc.dram_tensor(n, [2 * DH, B], FP8, kind="ExternalInput").ap()
           for n in NAMES]
    out_part = nc.dram_tensor("part", [128, 4], F32, kind="ExternalOutput").ap()
    out_rsab = nc.dram_tensor("rsab", [128, N_RSAB], F32,
                              kind="ExternalOutput").ap()
    # rect colsums: per sub 4 chunks of 512 (rolled cols 0..2047, DR-stacked
    # over both row-tiles); diag: per (fh, t) 2 chunks of 512 covering local
    # cols 128t+128 .. 128t+1151
    out_csr = nc.dram_tensor("csr", [9, 4, 512], BF16,
                             kind="ExternalOutput").ap()
    out_csd = nc.dram_tensor("csd", [12, 2, 2, 512], BF16,
                             kind="ExternalOutput").ap()

    with tile.TileContext(nc) as tc, ExitStack() as ctx:
        znt_pool = ctx.enter_context(tc.tile_pool(name="zntp", bufs=12))
        raw_pool = ctx.enter_context(tc.tile_pool(name="rawp", bufs=2))
        sq_pool = ctx.enter_context(tc.tile_pool(name="sqp", bufs=bbufs))
        zb_pool = ctx.enter_context(tc.tile_pool(name="zbp", bufs=2))
        bc_pool = ctx.enter_context(tc.tile_pool(name="bcp", bufs=bbufs))
        lgn_pool = ctx.enter_context(tc.tile_pool(name="lgnp", bufs=bbufs))
        es_pool = ctx.enter_context(tc.tile_pool(name="esp", bufs=3))
        cs_pool = ctx.enter_context(tc.tile_pool(name="csp", bufs=4))
        dsc_pool = ctx.enter_context(tc.tile_pool(name="dscp", bufs=2))
        acc_pool = ctx.enter_context(tc.tile_pool(name="accp", bufs=1))
        psum_pool = ctx.enter_context(
            tc.tile_pool(name="psump", bufs=2, space="PSUM"))

        load_eng = {"gpsimd": nc.gpsimd, "scalar": nc.scalar,
                    "sync": nc.sync}[loads_on]
        dots_eng = {"gpsimd": nc.gpsimd, "vector": nc.vector}[dots_on]

        biasm5 = acc_pool.tile([128, 1], F32, tag="biasm5", name="biasm5")
        nc.gpsimd.memset(biasm5[:], -5.0)
        biasln = acc_pool.tile([128, 1], F32, tag="biasln", name="biasln")
        nc.gpsimd.memset(biasln[:], LN_SCALE)
        # all-ones fp8 weights [K=128, P=128] -> replicated partition sums
        ones128 = acc_pool.tile([128, 128], FP8, tag="ones128", name="ones128")
        nc.gpsimd.memset(ones128[:], 1.0)
        # ones for DR colsums: lhsT [Ki=128, Ko=2, P] (Ko step must be
        # 16B-aligned, so allocate P=16 and slice P=1)
        ones_dr = acc_pool.tile([128, 2, 128], FP8, tag="onesdr",
                                name="ones_dr")
        nc.gpsimd.memset(ones_dr[:], 1.0)
        ones_1 = acc_pool.tile([128, 1], FP8, tag="ones1", name="ones_1")
        nc.gpsimd.memset(ones_1[:], 1.0)
        ones_col = acc_pool.tile([1, 128], BF16, tag="onesc", name="ones_col")
        nc.gpsimd.memset(ones_col[:], 1.0)

        rsab = acc_pool.tile([128, N_RSAB], F32, tag="rsab", name="rsab_sb")
        dots_all = acc_pool.tile([128, 21], F32, tag="dots", name="dots_all")
        part = acc_pool.tile([128, 4], F32, tag="part", name="part_sb")

        znt = {}
        sq_i = [0]
        bstage = 9
        if timing_mode.startswith("builds") and timing_mode[6:].isdigit():
            bstage = int(timing_mode[6:])
        # fixed rsab column layout (host combine depends on it):
        # diag (fh, t): cols 4*fh + 2*t + {0:A, 1:B}; rect (s, t):
        # cols 48 + 4*s + 2*t + {0, 1}
        RECT0 = 48

        def build_ff(fh):
            """Load raw fp8 XT half, normalize in place -> znt = (16*Zn)^T."""
            f, h = fh // 2, fh % 2
            zt = (raw_pool if mult_mode == "dmacast" else znt_pool).tile(
                [128, KC, B], FP8, tag="znt", name=f"znt{fh}")
            load_eng.dma_start(
                out=zt[:],
                in_=ffs[f][h * DH:(h + 1) * DH, :].rearrange(
                    "(kc p) n -> p kc n", p=128))
            sq = sq_pool.tile([128, KC, B], FP8, tag="sq", name=f"sq{fh}")
            if bstage < 1:
                znt[fh] = zt
                return
            # squares engine per sq_pat cycle ("A"=ACT, "D"=DVE)
            if sq_pat[sq_i[0] % len(sq_pat)] == "A":
                nc.scalar.activation(sq[:], zt[:], AF.Square)
            else:
                nc.vector.scalar_tensor_tensor(
                    out=sq[:], in0=zt[:], scalar=1.0, in1=zt[:],
                    op0=ALU.mult, op1=ALU.mult)
            sq_i[0] += 1
            bc = bc_pool.tile([128, B], BF16, tag="bc", name=f"bc{fh}")
            if bstage < 2:
                nc.vector.memset(bc[:, 0:2], 1.0)
                znt[fh] = zt
                return
            if norm_path == "small":
                # norms -> [1,B] bf16 row -> DRAM roundtrip -> [128,16] ->
                # tiny Ln/Exp -> [1,B] rinv row -> K=1 broadcast matmul
                nrow = lgn_pool.tile([1, B], BF16, tag="nrow",
                                     name=f"nr{fh}")
                for rnd in range(4):
                    nps = psum_pool.tile([128, 512], F32, tag="normp",
                                         bufs=2, name=f"nps{fh}_{rnd}")
                    cs = slice(rnd * 512, (rnd + 1) * 512)
                    for kc in range(KC):
                        nc.tensor.matmul(nps[0:1, :], ones_1[:],
                                         sq[:, kc, cs],
                                         start=(kc == 0),
                                         stop=(kc == KC - 1))
                    nc.vector.tensor_copy(nrow[:, cs], nps[0:1, :])
                nrow_d = nc.dram_tensor(f"nrow_d{fh}", [B], BF16,
                                        kind="Internal").ap()
                rinv_d = nc.dram_tensor(f"rinv_d{fh}", [128, B // 128],
                                        BF16, kind="Internal").ap()
                nc.sync.dma_start(out=nrow_d.rearrange("(a b) -> a b", a=1),
                                  in_=nrow[:])
                n128 = lgn_pool.tile([128, B // 128], BF16, tag="n128",
                                     name=f"n128_{fh}")
                nc.sync.dma_start(
                    out=n128[:], in_=nrow_d.rearrange("(p c) -> p c", p=128))
                lgn = lgn_pool.tile([128, B // 128], F32, tag="lgn",
                                    name=f"lgn{fh}")
                nc.scalar.activation(lgn[:], n128[:], AF.Ln)
                rinv = lgn_pool.tile([128, B // 128], BF16, tag="rinv",
                                     name=f"ri{fh}")
                nc.scalar.activation(rinv[:], lgn[:], AF.Exp, scale=-0.5,
                                     bias=biasln[:])
                nc.scalar.dma_start(out=rinv_d, in_=rinv[:])
                rrow = lgn_pool.tile([1, B], BF16, tag="rrow",
                                     name=f"rr{fh}")
                nc.scalar.dma_start(
                    out=rrow[:],
                    in_=rinv_d.rearrange("p c -> (p c)").rearrange(
                        "(a b) -> a b", a=1))
                for cb in range(4):
                    bps = psum_pool.tile([128, 512], F32, tag="normp",
                                         bufs=2, name=f"bps{fh}_{cb}")
                    cs = slice(cb * 512, (cb + 1) * 512)
                    nc.tensor.matmul(bps[:], ones_col[:], rrow[:, cs],
                                     start=True, stop=True)
                    nc.vector.tensor_copy(bc[:, cs], bps[:])
            else:
                for rnd in range(2):   # 2 x [128, 2, 512] PSUM rounds
                    nps = psum_pool.tile([128, 2, 512], F32, tag="normp",
                                         bufs=1, name=f"nps{fh}_{rnd}")
                    for cbl in range(2):
                        cs = slice((2 * rnd + cbl) * 512,
                                   (2 * rnd + cbl + 1) * 512)
                        for kc in range(KC):
                            nc.tensor.matmul(nps[:, cbl, :], ones128[:],
                                             sq[:, kc, cs],
                                             start=(kc == 0),
                                             stop=(kc == KC - 1))
                    lgn = lgn_pool.tile([128, 2, 512], F32, tag="lgn",
                                        name=f"lgn{fh}_{rnd}")
                    nc.scalar.activation(lgn[:], nps[:], AF.Ln)
                    # bc = exp(-0.5*ln(n) + ln16) = 16/sqrt(n), replicated
                    nc.scalar.activation(bc[:, rnd * 1024:(rnd + 1) * 1024],
                                         lgn[:], AF.Exp, scale=-0.5,
                                         bias=biasln[:])
            if bstage < 3:
                znt[fh] = zt
                return
            if mult_mode == "dmacast":
                # bf16-out multiply (fast DVE path) + SWDGE cast to fp8
                zb = zb_pool.tile([128, KC, B], BF16, tag="zb",
                                  name=f"zb{fh}")
                for kc in range(KC):
                    nc.vector.scalar_tensor_tensor(
                        out=zb[:, kc, :], in0=zt[:, kc, :], scalar=1.0,
                        in1=bc[:], op0=ALU.mult, op1=ALU.mult)
                zn = znt_pool.tile([128, KC, B], FP8, tag="znt2",
                                   name=f"znn{fh}")
                nc.gpsimd.dma_start(out=zn[:], in_=zb[:])
                znt[fh] = zn
            elif mult_mode == "off":   # timing probe only (wrong numerics)
                znt[fh] = zt
            else:
                for kc in range(KC):   # in-place: znt = raw * bc
                    nc.vector.scalar_tensor_tensor(
                        out=zt[:, kc, :], in0=zt[:, kc, :], scalar=1.0,
                        in1=bc[:], op0=ALU.mult, op1=ALU.mult)
                znt[fh] = zt

        def diag(fh):
            """Self sub-Gram of half fh: 2 row-tiles x col offsets 0..8."""
            zt = znt[fh]
            for t in range(2):
                base = 128 * t
                rcol = 4 * fh + 2 * t
                es = es_pool.tile([128, 1152], FP8, tag="esd",
                                  name=f"esd{fh}_{t}")
                # chunk A: local cols [base, base+1024) -> offsets 0..7
                psA = psum_pool.tile([128, 2, 512], F32, tag="gram",
                                    bufs=gram_bufs, name=f"dA{fh}_{t}")
                for cbl in range(2):
                    cs = slice(base + cbl * 512, base + (cbl + 1) * 512)
                    for q in range(KC // 2):
                        nc.tensor.matmul(
                            psA[:, cbl, :],
                            zt[:, 2 * q:2 * q + 2, base:base + 128],
                            zt[:, 2 * q:2 * q + 2, cs],
                            perf_mode=mybir.MatmulPerfMode.DoubleRow,
                            start=(q == 0), stop=(q == KC // 2 - 1))
                nc.scalar.activation(
                    es[:, 0:1024].rearrange("p (a b) -> p a b", a=2),
                    psA[:], AF.Exp, bias=biasm5[:], scale=EXP_SCALE,
                    accum_out=rsab[:, rcol:rcol + 1])
                # chunk B: local cols [base+1024, base+1152) -> offset 8
                psB = psum_pool.tile([128, 2, 512], F32, tag="gram",
                                     bufs=gram_bufs, name=f"dB{fh}_{t}")
                for q in range(KC // 2):
                    nc.tensor.matmul(
                        psB[:, 0, 0:128],
                        zt[:, 2 * q:2 * q + 2, base:base + 128],
                        zt[:, 2 * q:2 * q + 2, base + 1024:base + 1152],
                        perf_mode=mybir.MatmulPerfMode.DoubleRow,
                        start=(q == 0), stop=(q == KC // 2 - 1))
                nc.scalar.activation(
                    es[:, 1024:1152], psB[:, 0, 0:128], AF.Exp,
                    bias=biasm5[:], scale=EXP_SCALE,
                    accum_out=rsab[:, rcol + 1:rcol + 2])
                # colsums over local cols [128, 1024): offsets 1..7 only
                # (self tile = rowsum; offset 8 counted by both rowsums)
                for ck, w in ((0, 512), (1, 384)):
                    cps = psum_pool.tile([128, 512], F32, tag="cs", bufs=2,
                                         name=f"dc{fh}_{t}_{ck}")
                    nc.tensor.matmul(cps[:, 0:w], ones128[:],
                                     es[:, 128 + ck * 512:128 + ck * 512 + w],
                                     start=True, stop=True)
                    csb = cs_pool.tile([128, 512], BF16, tag="csb",
                                       name=f"dcs{fh}_{t}_{ck}")
                    nc.vector.tensor_copy(csb[:, 0:w], cps[:, 0:w])
                    nc.sync.dma_start(
                        out=out_csd[fh, t, ck, 0:w].rearrange(
                            "(a b) -> a b", a=1),
                        in_=csb[0:1, 0:w])

        def rect(s):
            """Cross sub-Gram for pair s: rows = shard of a, cols = all of b.
            es kept per (chunk) stacked over both row-tiles for DR colsums."""
            a, b = PAIRS[s]
            za, zb = znt[a], znt[b]
            ess = []
            for t in range(2):
                base = 128 * t
                for half_c in range(2):   # 2 psum tiles of [128, 2, 512]
                    ps = psum_pool.tile([128, 2, 512], F32, tag="gram",
                                        bufs=gram_bufs, name=f"r{s}_{t}_{half_c}")
                    for cbl in range(2):
                        cb = half_c * 2 + cbl
                        for q in range(KC // 2):
                            nc.tensor.matmul(
                                ps[:, cbl, :],
                                za[:, 2 * q:2 * q + 2, base:base + 128],
                                zb[:, 2 * q:2 * q + 2,
                                   cb * 512:(cb + 1) * 512],
                                perf_mode=mybir.MatmulPerfMode.DoubleRow,
                                start=(q == 0), stop=(q == KC // 2 - 1))
                    if t == 0:
                        es = es_pool.tile([128, 2, 2, 512], FP8, tag="esr",
                                          name=f"esr{s}_{half_c}")
                        ess.append(es)
                    else:
                        es = ess[half_c]
                    rcol = RECT0 + 4 * s + 2 * t + half_c
                    nc.scalar.activation(
                        es[:, t, :, :], ps[:], AF.Exp, bias=biasm5[:],
                        scale=EXP_SCALE,
                        accum_out=rsab[:, rcol:rcol + 1])
            # DR colsums: contract over rows of both tiles (K=256)
            for ck in range(4):
                es = ess[ck // 2]
                cps = psum_pool.tile([128, 512], F32, tag="cs", bufs=2,
                                     name=f"rc{s}_{ck}")
                nc.tensor.matmul(
                    cps[:], ones_dr[:],
                    es[:, :, ck % 2, :],
                    perf_mode=mybir.MatmulPerfMode.DoubleRow,
                    start=True, stop=True)
                csb = cs_pool.tile([128, 512], BF16, tag="csb",
                                   name=f"rcs{s}_{ck}")
                nc.vector.tensor_copy(csb[:], cps[:])
                nc.sync.dma_start(
                    out=out_csr[s, ck, :].rearrange("(a b) -> a b", a=1),
                    in_=csb[0:1, :])

        def dots(col, X, Y):
            """dots_all[:, col] = per-partition sum over the 256-row shard
            of <Zn_X[i], Zn_Y[i]>."""
            o = dsc_pool.tile([128, KC, R], F32, tag="dsc", name=f"do{col}")
            dots_eng.scalar_tensor_tensor(
                out=o[:], in0=znt[X][:, :, 0:R], scalar=DSCALE,
                in1=znt[Y][:, :, 0:R], op0=ALU.mult, op1=ALU.mult,
                accum_out=dots_all[:, col:col + 1])

        if timing_mode == "grams":
            shared = []
            for sI in range(9):
                t = znt_pool.tile([128, KC, B], FP8, tag="znt",
                                  name=f"znts{sI}")
                nc.vector.memset(t[:, :, 0:2], 0.0)
                shared.append(t)
            for fh in range(12):
                znt[fh] = shared[fh % 9]
            def build_ff(fh):
                pass
        elif timing_mode.startswith("builds"):
            nc.vector.memset(rsab[:], 1.0)
            def diag(fh):
                pass
            def rect(s):
                pass

        rep_ctx = tc.For_i(0, repeat, 1) if repeat > 1 else None
        if rep_ctx is not None:
            rep_ctx.__enter__()

        if timing_mode == "grams":
            nc.vector.memset(rsab[:], 0.0)

        # Emission: halves in a pair-friendly order; a diag follows its
        # build; rects and dots as soon as both operands exist.  dots cols:
        # 0..8 contrastive (PAIRS order), 9..20 ortho (ORTHO order).
        rect_done = set()
        ortho_done = set()

        def emit_ready():
            for s, (a, b) in enumerate(PAIRS):
                if s not in rect_done and a in znt and b in znt:
                    rect(s)
                    dots(s, a, b)
                    rect_done.add(s)
            for o, (a, b) in enumerate(ORTHO):
                if o not in ortho_done and a in znt and b in znt:
                    dots(9 + o, a, b)
                    ortho_done.add(o)

        order = [0, 2, 4, 6, 8, 10, 1, 7, 3, 9, 5, 11]
        pend = []
        for fh in order:
            build_ff(fh)
            pend.append(fh)
            if len(pend) > lag:
                diag(pend.pop(0))
            emit_ready()
        for fh in pend:
            diag(fh)
            emit_ready()

        # ---- epilogue: dots partial sums ----
        nc.vector.memset(part[:], 0.0)
        nc.vector.tensor_reduce(part[:, 1:2], dots_all[:, 0:9],
                                axis=mybir.AxisListType.X, op=ALU.add)
        nc.vector.tensor_reduce(part[:, 2:3], dots_all[:, 9:21],
                                axis=mybir.AxisListType.X, op=ALU.add)
        nc.sync.dma_start(out=out_part, in_=part[:])
        nc.scalar.dma_start(out=out_rsab, in_=rsab[:])

        if rep_ctx is not None:
            rep_ctx.__exit__(None, None, None)

    nc.compile()
    return nc


_PROG = None


def _get_prog():
    global _PROG
    if _PROG is None:
        _PROG = build_program()
    return _PROG


def make_in_maps(inputs):
    f8 = ml_dtypes.float8_e4m3
    base = {n: np.ascontiguousarray(
                np.asarray(inputs[n], dtype=np.float32).T).astype(f8)
            for n in NAMES}
    in_maps = []
    for c in range(N_CORES):
        m = {}
        for n in NAMES:
            if c == 0:
                m[n] = base[n]
            else:
                m[n] = np.ascontiguousarray(np.roll(base[n], -R * c, axis=1))
        in_maps.append(m)
    return in_maps


def combine(results):
    """results: list of 8 dicts with part/rsab/csr/csd -> scalar loss."""
    S_diag_row = np.zeros((12, B))   # diag rowsums per half, global rows
    S_diag_col = np.zeros((12, B))
    S_rect_row = np.zeros((9, B))    # rows of pair's a-side
    S_rect_col = np.zeros((9, B))    # rows of pair's b-side
    tcc = toc = 0.0

    for c, r in enumerate(results):
        part = np.asarray(r["part"], dtype=np.float64)
        tcc += part[:, 1].sum()
        toc += part[:, 2].sum()
        rsab = np.asarray(r["rsab"], dtype=np.float64)   # [128, N_RSAB]
        csr = np.asarray(r["csr"], dtype=np.float64)     # [9, 4, 512]
        csd = np.asarray(r["csd"], dtype=np.float64)     # [12, 2, 2, 512]
        for fh in range(12):
            for t in range(2):
                col = 4 * fh + 2 * t
                rows = (np.arange(128) + 128 * t + R * c) % B
                S_diag_row[fh, rows] += rsab[:, col] + rsab[:, col + 1]
                # colsums cover local cols 128t+128 .. 128t+1023
                lc = 128 * t + 128 + np.arange(896)
                gc = (lc + R * c) % B
                S_diag_col[fh, gc] += np.concatenate(
                    [csd[fh, t, 0, :], csd[fh, t, 1, 0:384]])
        for s in range(9):
            for t in range(2):
                col = 48 + 4 * s + 2 * t
                rows = (np.arange(128) + 128 * t + R * c) % B
                S_rect_row[s, rows] += rsab[:, col] + rsab[:, col + 1]
            gc = (np.arange(B) + R * c) % B
            S_rect_col[s, gc] += csr[s].reshape(B)

    tl = 0.0
    for s, (a, b) in enumerate(PAIRS):
        Sa = S_diag_row[a] + S_diag_col[a] + S_rect_row[s]
        Sb = S_diag_row[b] + S_diag_col[b] + S_rect_col[s]
        tl += np.log(Sa - 1.0).sum() + np.log(Sb - 1.0).sum()

    n2 = float(2 * B)
    loss = (9 * 5.0 + 12.0) + tl / n2 - 10.0 * tcc / n2 - toc / float(B)
    return np.float32(loss)


def kernel(**inputs):
    nc = _get_prog()
    in_maps = make_in_maps(inputs)
    res = run_bass_kernel_spmd(nc, in_maps, list(range(N_CORES)))
    return combine([res.results[c] for c in range(N_CORES)])
